# revision 4
# baseline (speedup 1.0000x reference)
"""Trainium2 Bass kernel for nn_ActorCritic (2-layer GraphConv + dense head).

Self-contained: takes full inputs, shards across 8 NeuronCores internally,
returns the full output (actor[6], critic[1]).

Strategy (see sharding hint): nodes sharded 8 ways by dst (edge-cut);
node features flow through two gather-based SpMM passes:
  pass 1: a1 = norm_src * (x_norm @ W1)  (table, allgathered bf16)
          m1 = A @ a1, scaled by norm_dst via indicator matmuls on TensorE
          h1n = relu(m1 * norm_dst + b1) * norm_src -> table 2
  pass 2: s2 = A @ h1n (same edge schedule), h2 = (s2 @ W2) * nd + b2
  head:   z = relu(concat(inputs, h2.flat) @ Wc + bc); actor/critic.

The SpMM gather uses dma_gather across 4 SWDGE queues (4 Q7 pairs emit
descriptors concurrently, ~1.9 ns/edge). Per-edge rows land 128-per-column;
each column is segment-reduced by a TensorE matmul against a small
[128, WIN] indicator whose values are norm_dst, into a PSUM window that
advances ADV node-slots per column (static schedule; host packs nodes into
a per-core order that provably fits, padding ~15%).
"""
import math
from dataclasses import dataclass, field

import numpy as np
import ml_dtypes

BF16 = ml_dtypes.bfloat16


@dataclass
class Cfg:
    n: int = 50000          # total nodes
    n_cores: int = 8
    f_in: int = 2624
    h: int = 128            # hidden width (must be 128: 256B gather rows)
    n_srv: int = 6
    e: int = 1600000
    adv: int = 7            # node-slot advance per column
    win: int = 9            # indicator window width
    col: int = 128          # edge slots per column
    chunk_cols: int = 18    # columns per (chunk, half); auto-bumped if needed

    @property
    def npc(self):
        return self.n // self.n_cores

    @property
    def chunk_w(self):
        return self.adv * self.chunk_cols

    @property
    def n_ch(self):
        return math.ceil(self.npc / self.chunk_w)

    @property
    def n_xt(self):
        return math.ceil(self.npc / 128)

    @property
    def nfb(self):
        return math.ceil(self.f_in / 128)

    @property
    def n_inp(self):
        return self.n + self.n_srv + 1

    @property
    def nw(self):
        return math.ceil(math.ceil(self.n_inp / self.n_cores) / 128)


CFG_FULL = Cfg()


# ---------------------------------------------------------------- host plan

def _pack_cell(d, cfg):
    """Greedy-place node degree sequence d into the static column schedule.
    Returns list per node-slot of [(col, count), ...] or None if infeasible."""
    out = []
    colk = 0
    used = 0
    for s in range(len(d)):
        k_lo = max(0, -(-(s - cfg.win + 1) // cfg.adv))
        k_hi = min(cfg.chunk_cols - 1, s // cfg.adv)
        if colk < k_lo:
            colk, used = k_lo, 0
        rem = int(d[s])
        alloc = []
        while rem > 0:
            if colk > k_hi:
                return None
            take = min(rem, cfg.col - used)
            if take > 0:
                alloc.append((colk, take))
            used += take
            rem -= take
            if used == cfg.col:
                colk += 1
                used = 0
        out.append(alloc)
    return out


def _order_chunk(nodes, tot, rng):
    nn = nodes[np.argsort(tot[nodes], kind="stable")]
    return np.concatenate([nn[0::2], nn[1::2][::-1]])


def _core_plan(deg_a, deg_b, cfg, rng):
    """Choose node order for one core; returns (perm, packsA, packsB) where
    packs* is a list per chunk of per-slot allocations."""
    npc = cfg.npc
    tot = deg_a + deg_b
    order = np.argsort(-tot, kind="stable")
    n_ch = cfg.n_ch
    caps = [cfg.chunk_w] * (n_ch - 1) + [npc - cfg.chunk_w * (n_ch - 1)]
    chunks = [[] for _ in range(n_ch)]
    i = 0
    for nd in order:
        placed = False
        for _ in range(2 * n_ch):
            c = i % (2 * n_ch)
            c = c if c < n_ch else 2 * n_ch - 1 - c
            i += 1
            if len(chunks[c]) < caps[c]:
                chunks[c].append(nd)
                placed = True
                break
        if not placed:
            for cc in range(n_ch):
                if len(chunks[cc]) < caps[cc]:
                    chunks[cc].append(nd)
                    break
    perm = []
    packs_a, packs_b = [], []
    for c in range(n_ch):
        nodes = np.asarray(chunks[c], dtype=np.int64)
        attempt = _order_chunk(nodes, tot, rng)
        pa = _pack_cell(deg_a[attempt], cfg)
        pb = _pack_cell(deg_b[attempt], cfg)
        tries = 0
        while (pa is None or pb is None) and tries < 400:
            shuf = attempt.copy()
            rng.shuffle(shuf)
            attempt = _order_chunk(shuf, tot, rng) if tries < 100 else shuf
            pa = _pack_cell(deg_a[attempt], cfg)
            pb = _pack_cell(deg_b[attempt], cfg)
            tries += 1
        if pa is None or pb is None:
            return None
        perm.extend(attempt.tolist())
        packs_a.append(pa)
        packs_b.append(pb)
    return np.asarray(perm, dtype=np.int64), packs_a, packs_b


def _wrap16(idx_flat):
    """[S] -> [128, S/16] int16: position i -> (i%16, i//16), replicated x8."""
    s = idx_flat.shape[0]
    w = idx_flat.reshape(s // 16, 16).T.astype(np.int16)
    return np.tile(w, (8, 1))


def build_plan(src, dst, cfg):
    """Full host preprocessing. Returns (plan dict per core list, cfg)."""
    n, npc = cfg.n, cfg.npc
    deg_out = np.bincount(src, minlength=n).astype(np.float64)
    deg_in = np.bincount(dst, minlength=n).astype(np.float64)
    norm_src = (1.0 / np.sqrt(np.maximum(deg_out, 1.0))).astype(np.float32)
    norm_dst = (1.0 / np.sqrt(np.maximum(deg_in, 1.0))).astype(np.float32)

    src_core = src // npc
    dst_core = dst // npc
    half_b = src_core >= (cfg.n_cores // 2)

    cores = []
    rng = np.random.default_rng(12345)
    for c in range(cfg.n_cores):
        sel = dst_core == c
        s_c = src[sel]
        d_loc = dst[sel] - c * npc
        h_c = half_b[sel]
        deg_a = np.bincount(d_loc[~h_c], minlength=npc)
        deg_b = np.bincount(d_loc[h_c], minlength=npc)
        res = None
        while res is None:
            res = _core_plan(deg_a, deg_b, cfg, rng)
            if res is None:
                cfg.chunk_cols += 1  # global static bump; rebuild all cores
                return build_plan(src, dst, cfg)
        perm, packs_a, packs_b = res
        cores.append(dict(perm=perm, packs_a=packs_a, packs_b=packs_b,
                          sel=sel, s_c=s_c, d_loc=d_loc, h_c=h_c))

    # permuted global row of original node g: row[g]
    row_of = np.empty(n, dtype=np.int64)
    for c in range(cfg.n_cores):
        row_of[c * npc + cores[c]["perm"]] = c * npc + np.arange(npc)

    n_ch, ccols, win, col = cfg.n_ch, cfg.chunk_cols, cfg.win, cfg.col
    cols_per_chunk = 2 * ccols
    tot_cols = n_ch * cols_per_chunk
    s_tot = tot_cols * col

    plans = []
    for c in range(cfg.n_cores):
        cc = cores[c]
        perm = cc["perm"]
        # edges grouped by (slot, half): slot of local dst
        inv_perm = np.empty(npc, dtype=np.int64)
        inv_perm[perm] = np.arange(npc)
        slot = inv_perm[cc["d_loc"]]
        rows = row_of[cc["s_c"]]  # permuted global row of source
        idx_buf = np.zeros(s_tot, dtype=np.int64)
        ind = np.zeros((col, tot_cols, win), dtype=np.float32)
        for half, packs in ((0, cc["packs_a"]), (1, cc["packs_b"])):
            m = cc["h_c"] == bool(half)
            e_slot = slot[m]
            e_row = rows[m] - (half * (n // 2))
            order = np.argsort(e_slot, kind="stable")
            e_slot = e_slot[order]
            e_row = e_row[order]
            ptr = 0
            for ch in range(n_ch):
                pk = packs[ch]
                colbase = ch * cols_per_chunk + half * ccols
                fill = np.zeros(ccols, dtype=np.int64)
                for s_in_chunk, alloc in enumerate(pk):
                    g_slot = ch * cfg.chunk_w + s_in_chunk
                    nd_val = norm_dst[c * npc + perm[g_slot]]
                    for (k, cnt) in alloc:
                        assert np.all(e_slot[ptr:ptr + cnt] == g_slot)
                        pos = fill[k]
                        gcol = colbase + k
                        idx_buf[gcol * col + pos: gcol * col + pos + cnt] = \
                            e_row[ptr:ptr + cnt]
                        w = s_in_chunk - cfg.adv * k
                        assert 0 <= w < win
                        ind[pos:pos + cnt, gcol, w] = nd_val
                        fill[k] += cnt
                        ptr += cnt
            assert ptr == e_slot.shape[0]
        plans.append(dict(
            idx=_wrap16(idx_buf),
            ind=ind.reshape(col, tot_cols * win).astype(BF16),
            perm=perm,
        ))

    meta = dict(norm_src=norm_src, norm_dst=norm_dst, row_of=row_of)
    return plans, meta


def build_inputs(inp, cfg, plans, meta):
    """Build per-core in_maps from the full input dict."""
    n, npc, h, nsrv = cfg.n, cfg.npc, cfg.h, cfg.n_srv
    x = np.asarray(inp["x"], np.float32)
    invec = np.asarray(inp["inputs"], np.float32)
    W1 = np.asarray(inp["W1"], np.float32)
    b1 = np.asarray(inp["b1"], np.float32)
    W2 = np.asarray(inp["W2"], np.float32)
    b2 = np.asarray(inp["b2"], np.float32)
    Wc = np.asarray(inp["Wc"], np.float32)
    bc = np.asarray(inp["bc"], np.float32)
    Wa = np.asarray(inp["Wa"], np.float32)
    ba = np.asarray(inp["ba"], np.float32)
    Wv = np.asarray(inp["Wv"], np.float32)
    bv = np.asarray(inp["bv"], np.float32)
    norm_src = meta["norm_src"]
    norm_dst = meta["norm_dst"]
    nd_bf = norm_dst.astype(BF16).astype(np.float32)
    nd_corr = np.where(nd_bf > 0, norm_dst / np.maximum(nd_bf, 1e-30), 1.0)

    n_inp = cfg.n_inp
    rows_per_core = math.ceil(n_inp / cfg.n_cores)
    tw = cfg.nw * 128
    wav = np.concatenate([Wa, Wv], axis=1).astype(np.float32)  # [h, 7]
    bav = np.concatenate([ba, bv])[None, :].astype(np.float32)  # [1, 7]
    id128 = np.eye(128, dtype=BF16)
    id6 = np.eye(nsrv, dtype=np.float32)
    id1 = np.eye(1, dtype=np.float32)

    in_maps = []
    for c in range(cfg.n_cores):
        perm = plans[c]["perm"]
        g = c * npc + perm  # original node ids in slot order
        ns_p = norm_src[g]
        # wrapped per x-tile [128, n_xt] and per chunk [128, n_ch]
        nsx = np.zeros((128, cfg.n_xt), np.float32)
        for t in range(cfg.n_xt):
            r = min(128, npc - t * 128)
            nsx[:r, t] = ns_p[t * 128: t * 128 + r]
        # fold the bf16(nd) rounding correction into the per-node scales
        corr_p = nd_corr[g]
        nsc = np.zeros((128, cfg.n_ch), np.float32)
        c2w = np.zeros((128, cfg.n_ch), np.float32)
        for ch in range(cfg.n_ch):
            r = min(cfg.chunk_w, npc - ch * cfg.chunk_w)
            sl = slice(ch * cfg.chunk_w, ch * cfg.chunk_w + r)
            nsc[:r, ch] = ns_p[sl] * corr_p[sl]
            c2w[:r, ch] = corr_p[sl]
        # Wc bottom rows for this core's slots (flattened reference order)
        base = n_inp
        rws = (base + 6 * g[:, None] + np.arange(nsrv)[None, :]).reshape(-1)
        wcb = Wc[rws].astype(np.float32)  # [npc*6, h]
        # inputs head slice
        lo = c * rows_per_core
        hi = min(n_inp, lo + rows_per_core)
        inpw = np.zeros(tw, np.float32)
        wct = np.zeros((tw, h), np.float32)
        if hi > lo:
            inpw[:hi - lo] = invec[lo:hi]
            wct[:hi - lo] = Wc[lo:hi]
        inpw = inpw.reshape(cfg.nw, 128).T.copy()  # [128, nw]
        in_maps.append({
            "x": x[g].astype(np.float32),
            "idx": plans[c]["idx"],
            "ind": plans[c]["ind"],
            "w1": W1,
            "w2b": W2.astype(BF16),
            "wcb": wcb,
            "wct": wct,
            "inpw": inpw,
            "nsx": nsx,
            "nsc": nsc,
            "c2w": c2w,
            "b2bc": np.tile(b2[None, :], (128, 1)).astype(np.float32),
            "b1c": b1.reshape(h, 1).astype(np.float32),
            "b2c": b2.reshape(nsrv, 1).astype(np.float32),
            "bcc": bc.reshape(h, 1).astype(np.float32),
            "bav": bav,
            "wav": wav,
            "idb": id128,
            "id6": id6,
            "id1": id1,
        })
    return in_maps


# ---------------------------------------------------------------- device code

def build_kernel(cfg, n_queues=4):
    import concourse.bacc as bacc
    import concourse.mybir as mybir
    import concourse.tile as tile

    f32 = mybir.dt.float32
    bf16 = mybir.dt.bfloat16
    i16 = mybir.dt.int16
    AF = mybir.ActivationFunctionType
    OP = mybir.AluOpType

    n, npc, h, nsrv = cfg.n, cfg.npc, cfg.h, cfg.n_srv
    n_ch, ccols, win, adv = cfg.n_ch, cfg.chunk_cols, cfg.win, cfg.adv
    cols_per_chunk = 2 * ccols
    tot_cols = n_ch * cols_per_chunk
    s_tot = tot_cols * cfg.col
    f_in, nfb, n_xt, nw = cfg.f_in, cfg.nfb, cfg.n_xt, cfg.nw

    nc = bacc.Bacc("TRN2", target_bir_lowering=False, debug=False,
                   enable_asserts=True, num_devices=cfg.n_cores,
                   num_swdge_queues=n_queues)

    x_ap = nc.dram_tensor("x", [npc, f_in], f32, kind="ExternalInput").ap()
    idx_ap = nc.dram_tensor("idx", [128, s_tot // 16], i16,
                            kind="ExternalInput").ap()
    ind_ap = nc.dram_tensor("ind", [128, tot_cols * win], bf16,
                            kind="ExternalInput").ap()
    w1_ap = nc.dram_tensor("w1", [f_in, h], f32, kind="ExternalInput").ap()
    w2b_ap = nc.dram_tensor("w2b", [h, nsrv], bf16, kind="ExternalInput").ap()
    wcb_ap = nc.dram_tensor("wcb", [npc * nsrv, h], f32,
                            kind="ExternalInput").ap()
    wct_ap = nc.dram_tensor("wct", [nw * 128, h], f32,
                            kind="ExternalInput").ap()
    inpw_ap = nc.dram_tensor("inpw", [128, nw], f32, kind="ExternalInput").ap()
    nsx_ap = nc.dram_tensor("nsx", [128, n_xt], f32, kind="ExternalInput").ap()
    nsc_ap = nc.dram_tensor("nsc", [128, n_ch], f32, kind="ExternalInput").ap()
    c2w_ap = nc.dram_tensor("c2w", [128, n_ch], f32, kind="ExternalInput").ap()
    b2bc_ap = nc.dram_tensor("b2bc", [128, nsrv], f32,
                             kind="ExternalInput").ap()
    b1c_ap = nc.dram_tensor("b1c", [h, 1], f32, kind="ExternalInput").ap()
    b2c_ap = nc.dram_tensor("b2c", [nsrv, 1], f32, kind="ExternalInput").ap()
    bcc_ap = nc.dram_tensor("bcc", [h, 1], f32, kind="ExternalInput").ap()
    bav_ap = nc.dram_tensor("bav", [1, 7], f32, kind="ExternalInput").ap()
    wav_ap = nc.dram_tensor("wav", [h, 7], f32, kind="ExternalInput").ap()
    idb_ap = nc.dram_tensor("idb", [128, 128], bf16, kind="ExternalInput").ap()
    id6_ap = nc.dram_tensor("id6", [nsrv, nsrv], f32,
                            kind="ExternalInput").ap()
    id1_ap = nc.dram_tensor("id1", [1, 1], f32, kind="ExternalInput").ap()
    out_ap = nc.dram_tensor("out", [1, 7], f32, kind="ExternalOutput").ap()

    rg = [list(range(cfg.n_cores))]

    with tile.TileContext(nc) as tc:
        with (
            tc.tile_pool(name="consts", bufs=1) as cp,
            tc.tile_pool(name="xio", bufs=2) as xio,
            tc.tile_pool(name="trp", bufs=2) as trp,
            tc.tile_pool(name="gath", bufs=3) as gp,
            tc.tile_pool(name="work", bufs=3) as wp,
            tc.tile_pool(name="ps_y", bufs=2, space="PSUM") as ps_y,
            tc.tile_pool(name="ps_tr", bufs=2, space="PSUM") as ps_tr,
            tc.tile_pool(name="ps_mm", bufs=2, space="PSUM") as ps_mm,
            tc.tile_pool(name="ps_z", bufs=1, space="PSUM") as ps_z,
            tc.tile_pool(name="ps_s", bufs=1, space="PSUM") as ps_s,
            tc.tile_pool(name="dram", bufs=1, space="DRAM") as dp,
        ):
            # ---------------- consts
            idx_sb = cp.tile([128, s_tot // 16], i16)
            nc.sync.dma_start(idx_sb[:], idx_ap[:])
            ind_sb = cp.tile([128, tot_cols, win], bf16)
            nc.sync.dma_start(
                ind_sb.rearrange("p a b -> p (a b)")[:], ind_ap[:])
            w1_sb = cp.tile([128, nfb * h], bf16)
            for fb in range(nfb):
                r = min(128, f_in - fb * 128)
                nc.gpsimd.dma_start(w1_sb[:r, fb * h:(fb + 1) * h],
                                    w1_ap[fb * 128: fb * 128 + r, :])
            w2_sb = cp.tile([h, nsrv], bf16)
            nc.sync.dma_start(w2_sb[:], w2b_ap[:])
            inpw_sb = cp.tile([128, nw], f32)
            nc.sync.dma_start(inpw_sb[:], inpw_ap[:])
            nsx_sb = cp.tile([128, n_xt], f32)
            nc.sync.dma_start(nsx_sb[:], nsx_ap[:])
            nsc_sb = cp.tile([128, n_ch], f32)
            nc.sync.dma_start(nsc_sb[:], nsc_ap[:])
            c2w_sb = cp.tile([128, n_ch], f32)
            nc.sync.dma_start(c2w_sb[:], c2w_ap[:])
            b2bc_sb = cp.tile([128, nsrv], f32)
            nc.sync.dma_start(b2bc_sb[:], b2bc_ap[:])
            b1c = cp.tile([h, 1], f32)
            nc.sync.dma_start(b1c[:], b1c_ap[:])
            b2c = cp.tile([nsrv, 1], f32)
            nc.sync.dma_start(b2c[:], b2c_ap[:])
            bcc = cp.tile([h, 1], f32)
            nc.sync.dma_start(bcc[:], bcc_ap[:])
            bav = cp.tile([1, 7], f32)
            nc.sync.dma_start(bav[:], bav_ap[:])
            wav = cp.tile([h, 7], f32)
            nc.sync.dma_start(wav[:], wav_ap[:])
            idb = cp.tile([128, 128], bf16)
            nc.sync.dma_start(idb[:], idb_ap[:])
            id6 = cp.tile([nsrv, nsrv], f32)
            nc.sync.dma_start(id6[:], id6_ap[:])
            id1 = cp.tile([1, 1], f32)
            nc.sync.dma_start(id1[:], id1_ap[:])
            ones = cp.tile([128, 1], bf16)
            nc.vector.memset(ones[:], 1.0)

            # DRAM internals
            a1_local = dp.tile([npc, h], bf16)
            a1_full = dp.tile([n, h], bf16)
            h1_local = dp.tile([npc, h], bf16)
            h1_full = dp.tile([n, h], bf16)
            mm_in = dp.tile([128, 2], f32)
            mm_out = dp.tile([128, 2], f32)
            z_in = dp.tile([1, h], f32)
            z_out = dp.tile([1, h], f32)

            # ---------------- z psum: head contributions accumulate here
            zp = ps_z.tile([1, h], f32)
            nc.vector.memset(zp[:], 0.0)

            # inputs-head part: z += inputs_slice @ Wct
            for k in range(nw):
                wt = wp.tile([128, h], f32, tag="wct")
                nc.sync.dma_start(wt[:], wct_ap[k * 128:(k + 1) * 128, :])
                nc.tensor.matmul(zp[:], inpw_sb[:, k:k + 1], wt[:],
                                 start=False, stop=False,
                                 skip_group_check=True)

            # ---------------- x phase: yT = (x @ W1)^T, min/max of x
            yT = cp.tile([128, npc], f32)
            mx = cp.tile([128, 1], f32)
            mn = cp.tile([128, 1], f32)
            nc.vector.memset(mx[:], -1e30)
            nc.vector.memset(mn[:], 1e30)
            cw1p = ps_y.tile([128, 1], f32, tag="yp")
            for t in range(n_xt):
                r = min(128, npc - t * 128)
                xf = xio.tile([128, f_in], f32, tag="xf")
                nc.sync.dma_start(xf[:r, :], x_ap[t * 128: t * 128 + r, :])
                red = wp.tile([128, 1], f32, tag="red")
                nc.vector.tensor_reduce(red[:r], xf[:r, :],
                                        axis=mybir.AxisListType.X, op=OP.max)
                nc.vector.tensor_tensor(mx[:r], mx[:r], red[:r], op=OP.max)
                red2 = wp.tile([128, 1], f32, tag="red2")
                nc.vector.tensor_reduce(red2[:r], xf[:r, :],
                                        axis=mybir.AxisListType.X, op=OP.min)
                nc.vector.tensor_tensor(mn[:r], mn[:r], red2[:r], op=OP.min)
                xt_ = xio.tile([128, f_in], bf16, tag="xin")
                nc.vector.tensor_copy(xt_[:r, :], xf[:r, :])
                yp = ps_y.tile([128, 128], f32, tag="yp")
                for fb in range(nfb):
                    fr = min(128, f_in - fb * 128)
                    tp = ps_tr.tile([128, 128], bf16, tag="trp")
                    nc.tensor.transpose(tp[:fr, :r],
                                        xt_[:r, fb * 128: fb * 128 + fr],
                                        idb[:r, :r])
                    xts = trp.tile([128, 128], bf16, tag="xts")
                    nc.vector.tensor_copy(xts[:fr, :r], tp[:fr, :r])
                    nc.tensor.matmul(yp[:, :r], w1_sb[:fr, fb * h:(fb + 1) * h],
                                     xts[:fr, :r],
                                     start=(fb == 0), stop=(fb == nfb - 1))
                nc.vector.tensor_copy(yT[:, t * 128: t * 128 + r], yp[:, :r])

            # colsum of W1 (for the -min correction)
            for fb in range(nfb):
                fr = min(128, f_in - fb * 128)
                nc.tensor.matmul(cw1p[:], w1_sb[:fr, fb * h:(fb + 1) * h],
                                 ones[:fr, :],
                                 start=(fb == 0), stop=(fb == nfb - 1))
            cw1 = cp.tile([128, 1], f32)
            nc.vector.tensor_copy(cw1[:], cw1p[:])

            # global min/max via partition reduce + AllReduce(max)
            mmx = wp.tile([128, 1], f32, tag="mmx")
            nc.gpsimd.partition_all_reduce(mmx[:], mx[:], 128,
                                           _reduce_op_max())
            mnneg = wp.tile([128, 1], f32, tag="mnneg")
            nc.vector.tensor_scalar_mul(mnneg[:], mn[:], -1.0)
            mmn = wp.tile([128, 1], f32, tag="mmn")
            nc.gpsimd.partition_all_reduce(mmn[:], mnneg[:], 128,
                                           _reduce_op_max())
            mmsb = wp.tile([128, 2], f32, tag="mmsb")
            nc.vector.tensor_copy(mmsb[:, 0:1], mmx[:])
            nc.vector.tensor_copy(mmsb[:, 1:2], mmn[:])
            nc.sync.dma_start(mm_in[:], mmsb[:])
            nc.gpsimd.collective_compute(
                "AllReduce", mybir.AluOpType.max, replica_groups=rg,
                ins=[mm_in[:].opt()], outs=[mm_out[:].opt()])
            gmm = cp.tile([128, 2], f32)
            nc.sync.dma_start(gmm[:], mm_out[:])
            # scale = 1/(gmax + gminneg); mncn = gminneg * cw1
            rng_ = cp.tile([128, 1], f32)
            nc.vector.tensor_tensor(rng_[:], gmm[:, 0:1], gmm[:, 1:2],
                                    op=OP.add)
            scale = cp.tile([128, 1], f32)
            nc.vector.reciprocal(scale[:], rng_[:])
            mncn = cp.tile([128, 1], f32)
            nc.vector.tensor_tensor(mncn[:], gmm[:, 1:2], cw1[:], op=OP.mult)
            nsqx = cp.tile([128, n_xt], f32)
            nc.vector.tensor_scalar_mul(nsqx[:], nsx_sb[:], scale[:])

            # ---------------- a1 table: (yT + mncn) ^T * (scale*ns)
            for t in range(n_xt):
                r = min(128, npc - t * 128)
                tcor = trp.tile([128, 128], bf16, tag="tcor")
                nc.vector.tensor_scalar_add(tcor[:, :r],
                                            yT[:, t * 128: t * 128 + r],
                                            mncn[:])
                tp = ps_tr.tile([128, 128], bf16, tag="trp")
                nc.tensor.transpose(tp[:r, :], tcor[:, :r], idb[:, :])
                a1r = trp.tile([128, 128], bf16, tag="a1r")
                nc.vector.tensor_scalar_mul(a1r[:r, :], tp[:r, :],
                                            nsqx[:r, t:t + 1])
                nc.sync.dma_start(a1_local[t * 128: t * 128 + r, :],
                                  a1r[:r, :])
            nc.gpsimd.collective_compute(
                "AllGather", mybir.AluOpType.bypass, replica_groups=rg,
                ins=[a1_local[:].opt()], outs=[a1_full[:].opt()])

            # ---------------- SpMM passes
            def spmm_pass(table_full, post_fn, tag):
                tab_a = table_full[0: n // 2, :]
                tab_b = table_full[n // 2: n, :]
                n_idx = ccols * cfg.col
                for ch in range(n_ch):
                    w_c = min(cfg.chunk_w, npc - ch * cfg.chunk_w)
                    gts = []
                    for hf, tab in ((0, tab_a), (1, tab_b)):
                        gcol0 = ch * cols_per_chunk + hf * ccols
                        s16 = gcol0 * cfg.col // 16
                        gt = gp.tile([128, ccols, h], bf16, tag=f"g{hf}")
                        nc.gpsimd.dma_gather(
                            gt[:], tab[:],
                            idx_sb[:, s16: s16 + n_idx // 16],
                            n_idx, n_idx, h, single_packet=False,
                            queue_num=(ch * 2 + hf) % n_queues)
                        gts.append(gt)
                    mp = ps_mm.tile([128, 128], f32, tag="mp")
                    nc.vector.memset(mp[:], 0.0)
                    nmm = 0
                    for hf in (0, 1):
                        gcol0 = ch * cols_per_chunk + hf * ccols
                        gt = gts[hf]
                        for k in range(ccols):
                            o = adv * k
                            nmm += 1
                            nc.tensor.matmul(
                                mp[:, o:o + win], gt[:, k, :],
                                ind_sb[:, gcol0 + k, :],
                                start=False, stop=(nmm == cols_per_chunk),
                                skip_group_check=True)
                    post_fn(ch, w_c, mp)

            # pass 1 epilogue: h1n rows -> h1_local
            def post1(ch, w_c, mp):
                h1 = wp.tile([128, 128], bf16, tag="h1")
                nc.scalar.activation(h1[:, :w_c], mp[:, :w_c], AF.Relu,
                                     bias=b1c[:], scale=1.0)
                tp = ps_tr.tile([128, 128], bf16, tag="trp")
                nc.tensor.transpose(tp[:w_c, :], h1[:, :w_c], idb[:, :])
                h1n = wp.tile([128, 128], bf16, tag="h1n")
                nc.vector.tensor_scalar_mul(h1n[:w_c, :], tp[:w_c, :],
                                            nsc_sb[:w_c, ch:ch + 1])
                nc.sync.dma_start(
                    h1_local[ch * cfg.chunk_w: ch * cfg.chunk_w + w_c, :],
                    h1n[:w_c, :])

            spmm_pass(a1_full, post1, "1")
            nc.gpsimd.collective_compute(
                "AllGather", mybir.AluOpType.bypass, replica_groups=rg,
                ins=[h1_local[:].opt()], outs=[h1_full[:].opt()])

            # pass 2 epilogue: h2 -> z psum contributions
            def post2(ch, w_c, mp):
                t2 = wp.tile([128, 128], bf16, tag="t2")
                nc.vector.tensor_copy(t2[:, :w_c], mp[:, :w_c])
                h2p = ps_s.tile([nsrv, 128], f32, tag="small")
                nc.tensor.matmul(h2p[:, :w_c], w2_sb[:], t2[:, :w_c],
                                 start=True, stop=True)
                h2t = wp.tile([nsrv, 128], f32, tag="h2t")
                nc.vector.tensor_copy(h2t[:, :w_c], h2p[:, :w_c])
                h2tp = ps_tr.tile([128, nsrv], f32, tag="trp")
                nc.tensor.transpose(h2tp[:w_c, :], h2t[:, :w_c],
                                    id6[:, :])
                h2s = wp.tile([128, nsrv], f32, tag="h2s")
                nc.vector.tensor_scalar_mul(h2s[:w_c, :], h2tp[:w_c, :],
                                            c2w_sb[:w_c, ch:ch + 1])
                h2 = wp.tile([128, nsrv], f32, tag="h2")
                nc.vector.tensor_tensor(h2[:w_c, :], h2s[:w_c, :],
                                        b2bc_sb[:w_c, :], op=OP.add)
                wcbt = wp.tile([128, nsrv * h], f32, tag="wcbt")
                r0 = ch * cfg.chunk_w * nsrv
                nc.sync.dma_start(
                    wcbt[:w_c, :],
                    wcb_ap[r0: r0 + w_c * nsrv, :].rearrange(
                        "(a b) c -> a (b c)", b=nsrv))
                for s in range(nsrv):
                    nc.tensor.matmul(zp[:], h2[:w_c, s:s + 1],
                                     wcbt[:w_c, s * h:(s + 1) * h],
                                     start=False,
                                     stop=(ch == n_ch - 1 and s == nsrv - 1),
                                     skip_group_check=True)

            spmm_pass(h1_full, post2, "2")

            # ---------------- head
            zsb = wp.tile([1, h], f32, tag="zsb")
            nc.vector.tensor_copy(zsb[:], zp[:])
            nc.sync.dma_start(z_in[:], zsb[:])
            nc.gpsimd.collective_compute(
                "AllReduce", mybir.AluOpType.add, replica_groups=rg,
                ins=[z_in[:].opt()], outs=[z_out[:].opt()])
            zsb2 = wp.tile([1, h], f32, tag="zsb2")
            nc.sync.dma_start(zsb2[:], z_out[:])
            ztp = ps_tr.tile([h, 1], f32, tag="trp")
            nc.tensor.transpose(ztp[:], zsb2[:], id1[:, :])
            zcol = wp.tile([h, 1], f32, tag="zcol")
            nc.scalar.activation(zcol[:], ztp[:], AF.Relu, bias=bcc[:],
                                 scale=1.0)
            op7 = ps_s.tile([1, 7], f32, tag="small")
            nc.tensor.matmul(op7[:], zcol[:], wav[:], start=True, stop=True)
            osb = wp.tile([1, 7], f32, tag="osb")
            nc.vector.tensor_tensor(osb[:], op7[:], bav[:], op=OP.add)
            nc.sync.dma_start(out_ap[:], osb[:])

    nc.compile()
    return nc


def _reduce_op_max():
    import concourse.bass_isa as bass_isa
    return bass_isa.ReduceOp.max


# ---------------------------------------------------------------- entry point

def run_on_hw(cfg, in_maps, nc=None, n_time=0):
    """Compile (or reuse) and execute on 8 cores; returns (out_maps, t_ns)."""
    import jax
    import numpy as _np
    from jax.sharding import Mesh, PartitionSpec, NamedSharding
    from jax.experimental.shard_map import shard_map
    import concourse.mybir as mybir
    from concourse import bass2jax
    from concourse.bass2jax import _bass_exec_p, install_neuronx_cc_hook

    if nc is None:
        nc = build_kernel(cfg)
    install_neuronx_cc_hook()
    partition_name = (nc.partition_id_tensor.name
                      if nc.partition_id_tensor else None)
    in_names, out_names, out_avals, zero_outs = [], [], [], []
    for alloc in nc.m.functions[0].allocations:
        if not isinstance(alloc, mybir.MemoryLocationSet):
            continue
        name = alloc.memorylocations[0].name
        if alloc.kind == "ExternalInput":
            if name != partition_name:
                in_names.append(name)
        elif alloc.kind == "ExternalOutput":
            out_names.append(name)
            shape = tuple(alloc.tensor_shape)
            dtype = mybir.dt.np(alloc.dtype)
            out_avals.append(jax.core.ShapedArray(shape, dtype))
            zero_outs.append(_np.zeros(shape, dtype))
    n_params = len(in_names)
    n_outs = len(out_avals)
    in_names_all = in_names + out_names
    if partition_name is not None:
        in_names_all = in_names_all + [partition_name]
    donate = tuple(range(n_params, n_params + n_outs))

    def _body(*args):
        operands = list(args)
        if partition_name is not None:
            operands.append(bass2jax.partition_id_tensor())
        outs = _bass_exec_p.bind(
            *operands, out_avals=tuple(out_avals),
            in_names=tuple(in_names_all), out_names=tuple(out_names),
            lowering_input_output_aliases=(),
            sim_require_finite=True, sim_require_nnan=True, nc=nc)
        return tuple(outs)

    devices = jax.devices()[:cfg.n_cores]
    mesh = Mesh(_np.asarray(devices), ("core",))
    in_specs = (PartitionSpec("core"),) * (n_params + n_outs)
    out_specs = (PartitionSpec("core"),) * len(out_names)
    sharded = jax.jit(
        shard_map(_body, mesh=mesh, in_specs=in_specs, out_specs=out_specs,
                  check_rep=False),
        donate_argnums=donate, keep_unused=True)
    sharding = NamedSharding(mesh, PartitionSpec("core"))
    per_core = [[_np.asarray(m[nm]) for nm in in_names] for m in in_maps]
    concat_in = [
        _np.concatenate([per_core[c][i] for c in range(cfg.n_cores)], axis=0)
        for i in range(n_params)]
    concat_in = [jax.device_put(a, sharding) for a in concat_in]

    def zeros():
        return [jax.device_put(
            _np.zeros((cfg.n_cores * z.shape[0], *z.shape[1:]), z.dtype),
            sharding) for z in zero_outs]

    out_arrs = sharded(*concat_in, *zeros())
    jax.block_until_ready(out_arrs)
    out_np = [_np.asarray(o) for o in out_arrs]
    t_ns = None
    if n_time:
        import time
        times = []
        for _ in range(n_time):
            zs = zeros()
            jax.block_until_ready(zs)
            t0 = time.perf_counter()
            o = sharded(*concat_in, *zs)
            jax.block_until_ready(o)
            times.append(time.perf_counter() - t0)
        t_ns = int(min(times) * 1e9)
    out_maps = [
        {nm: out_np[i].reshape(cfg.n_cores, *out_avals[i].shape)[c]
         for i, nm in enumerate(out_names)}
        for c in range(cfg.n_cores)]
    return out_maps, t_ns


_COMPILED = {}


def kernel(**inputs):
    cfg = Cfg()
    src = np.asarray(inputs["src"], np.int64)
    dst = np.asarray(inputs["dst"], np.int64)
    plans, meta = build_plan(src, dst, cfg)
    in_maps = build_inputs(inputs, cfg, plans, meta)
    key = "full"
    if key not in _COMPILED:
        _COMPILED[key] = build_kernel(cfg)
    out_maps, _ = run_on_hw(cfg, in_maps, nc=_COMPILED[key])
    out = out_maps[0]["out"][0]  # [7]
    actor = out[:6].astype(np.float32)
    critic = out[6:7].astype(np.float32)
    return actor, critic
